# revision 21
# baseline (speedup 1.0000x reference)
"""Trainium2 Bass kernel for a 1D Kernel Neural Operator (KNO) on a regular grid.

Reference computation (N=2048 nodes, C=32 channels, DEPTH=3):
    fq = gelu([f_x, x] @ lift_W.T + lift_b)
    for i in 0..2:
        skip  = fq @ pw_W[i].T + pw_b[i]
        K_c   = sig2_c * exp(-(x_n - x_q)^2 * a_c),  a_c = 1/(2*ell2_c)
        integ = einsum('cnq,qc->nc', K, fq * w)
        fq    = skip + integ; gelu if i < 2
    out = (gelu(gelu(fq@W1.T+b1)@W2.T+b2)) @ W3.T + b3

Instead of materializing the C x N x N kernels, we use the factorization
exp(-a(x_n-x_q)^2) = e^{-a x_n^2} e^{2 a x_n x_q} e^{-a x_q^2} and the Taylor
expansion e^{2a x_n x_q} = sum_k (2a)^k/k! x_n^k x_q^k (K=32 terms; the
neglected tail is < ~1e-5 relative for the a-range this problem generates).
Each layer's integral becomes two small matmuls through the moment basis:
    U      = fq ⊙ e^{-a_c x_q^2}                     [N,C]
    M[k,c] = sum_q (w_q x_q^k) U[q,c]               [K,C]   (PE matmul)
    Mt     = M ⊙ B,  B[k,c] = (2a_c)^k/k!
    integ  = (s2_c e^{-a_c x_n^2}) ⊙ (V @ Mt)       [N,C]   (PE matmul)

All x-grid/parameter-derived constants (moment basis V, its transpose, the
exponential prefactor tensors, scaled weights) are precomputed host-side in
float64 and shipped in three batched DMA blocks, so the device program is just
the f_x-dependent dataflow: ~90 engine instructions, all matmuls in bf16
(single-pass PE mode, fp32 PSUM accumulate), gelu the only activation function.

Data layout: the [N,C] state lives channel-transposed in a 4-chunk stack
fqT4[c + 32j, n'] = fq[512j + n', c], a single [128, 512] SBUF tile. Channel
mixing / eval / final projection each run as 4 concurrent quadrant matmuls.

Sharding: the whole problem is a ~30us chain of dependent small ops, so all 8
cores run identical replicas (collectives would cost more than they save); the
output is taken from core 0.
"""

import numpy as np

import concourse.bass as bass
import concourse.tile as tile
from concourse import bacc, mybir
from concourse.bass_utils import run_bass_kernel_spmd

N = 2048
C = 32
K = 32
DEPTH = 3
NCORES = 8
NT = N // 128             # 16 q-tiles of 128
NCHUNK = N // 512         # 4 chunks of 512
F32 = mybir.dt.float32
BF16 = mybir.dt.bfloat16
NPBF16 = mybir.dt.np(mybir.dt.bfloat16)
AF = mybir.ActivationFunctionType
ALU = mybir.AluOpType

# pack1 column offsets
P_IDENT = 0
P_V4 = P_IDENT + 128          # V4[p, 32t+k] = w_q x_q^k, q = 128t+p
P_PWW = P_V4 + NT * K         # pwWT3[c+32j, 32i+c'] = pw_W[i][c',c]
P_P1W = P_PWW + 3 * C
P_P2W = P_P1W + C
P_B4 = P_P2W + C              # B4_3[32s+k, 32i+c] = (2a_ic)^k/k!
P_P3W = P_B4 + 3 * C
P_PWB = P_P3W + 4             # pwb3[c+32j, i] = pw_b[i][c]
P_P1B = P_PWB + 3
P_P2B = P_P1B + 1
P_P3B = P_P2B + 1
P_LB = P_P3B + 1              # liftb4[c+32j] = lift_b[c]
P_LW = P_LB + 1               # liftW[r, c] = lift_W[c, r], rows 0..1
W1 = P_LW + C
WC = 8                        # packc (f32): pwb3 | p1b | p2b | p3b | liftb
PC_PWB = 0
PC_P1B = 3
PC_P2B = 4
PC_P3B = 5
PC_LB = 6
W2 = 1024                     # pack2: tmpN_0 | Es_0
W3 = 4 * 512                  # pack3: tmpN_1 | Es_1 | tmpN_2 | Es_2

_CACHE = {}


def build_program(nc):
    pack1 = nc.dram_tensor("pack1", [128, W1], BF16, kind="ExternalInput")
    pack2 = nc.dram_tensor("pack2", [128, W2], BF16, kind="ExternalInput")
    pack3 = nc.dram_tensor("pack3", [128, W3], BF16, kind="ExternalInput")
    lift_in = nc.dram_tensor("lift_in", [2, N], BF16, kind="ExternalInput")
    vt_dram = nc.dram_tensor("vt", [K, N], BF16, kind="ExternalInput")
    packc = nc.dram_tensor("packc", [128, WC], F32, kind="ExternalInput")
    out_dram = nc.dram_tensor("out", [1, N], F32, kind="ExternalOutput")

    with tile.TileContext(nc) as tc:
        with (
            tc.tile_pool(name="const", bufs=1) as cp,
            tc.tile_pool(name="work", bufs=2) as wp,
            tc.tile_pool(name="psA", bufs=2, space="PSUM") as ppa,
            tc.tile_pool(name="psB", bufs=2, space="PSUM") as ppb,
            tc.tile_pool(name="psC", bufs=2, space="PSUM") as ppc,
            tc.tile_pool(name="psS", bufs=1, space="PSUM") as pps,
        ):
            # ------- batched input loads on four different queues -------
            liftsb = cp.tile([2, N], BF16, tag="liftsb")
            nc.sync.dma_start(liftsb[:], lift_in[:])
            pc = cp.tile([128, WC], F32, tag="pc")
            nc.sync.dma_start(pc[:], packc[:])
            p1 = cp.tile([128, W1], BF16, tag="p1")
            nc.sync.dma_start(p1[:], pack1[:])
            p2 = cp.tile([128, W2], BF16, tag="p2")
            nc.gpsimd.dma_start(p2[:], pack2[:])
            vt = cp.tile([K, N], BF16, tag="vt")
            nc.gpsimd.dma_start(vt[:], vt_dram[:])
            p3 = cp.tile([128, W3], BF16, tag="p3")
            nc.sync.dma_start(p3[:], pack3[:])

            ident = p1[:, P_IDENT : P_IDENT + 128]
            V4 = p1[:, P_V4 : P_V4 + NT * K]
            warmp = pps.tile([32, 512], F32, tag="warm")

            def warm(rhs):
                nc.tensor.matmul(
                    warmp[:],
                    p1[:, P_V4 : P_V4 + 32],
                    rhs,
                    start=True,
                    stop=True,
                    skip_group_check=True,
                )


            def tmpN(i):
                src = p2 if i == 0 else p3
                off = 0 if i == 0 else 1024 * (i - 1)
                return src[:, off : off + 512]

            def Es(i):
                src = p2 if i == 0 else p3
                off = 512 if i == 0 else 1024 * (i - 1) + 512
                return src[:, off : off + 512]

            # ---------------- lift ----------------
            liftp = ppa.tile([128, 512], F32, tag="mix")
            for j in range(NCHUNK):
                nc.tensor.matmul(
                    liftp[32 * j : 32 * (j + 1), :],
                    p1[0:2, P_LW : P_LW + C],
                    liftsb[:, 512 * j : 512 * (j + 1)],
                    start=True,
                    stop=True,
                    tile_position=(0, 32 * j),
                )
            fq = wp.tile([128, 512], BF16, tag="fq")
            nc.scalar.activation(
                fq[:], liftp[:], AF.Gelu_apprx_tanh, bias=pc[:, PC_LB : PC_LB + 1]
            )

            # ---------------- KNO layers ----------------
            for i in range(DEPTH):
                # transpose to natural layout first: U-mul (DVE) then overlaps
                # the skip matmuls (PE)
                trp = ppb.tile([128, 512], BF16, tag="trp")
                for m in range(4):
                    nc.tensor.transpose(
                        trp[:, 128 * m : 128 * (m + 1)],
                        fq[:, 128 * m : 128 * (m + 1)],
                        ident,
                    )
                skp = ppa.tile([128, 512], F32, tag="mix")
                for j in range(NCHUNK):
                    nc.tensor.matmul(
                        skp[32 * j : 32 * (j + 1), :],
                        p1[32 * j : 32 * (j + 1), P_PWW + C * i : P_PWW + C * (i + 1)],
                        fq[32 * j : 32 * (j + 1), :],
                        start=True,
                        stop=True,
                        tile_position=(32 * j, 32 * j),
                    )
                U = wp.tile([128, 512], BF16, tag="U")
                nc.vector.tensor_mul(U[:], trp[:], tmpN(i))
                # moments M[k,c] = sum_q (w_q x_q^k) U[q,c]
                Mp = pps.tile([K, C], F32, tag="Mp")
                for t in range(NT):
                    m, j = t % 4, t // 4
                    nc.tensor.matmul(
                        Mp[:],
                        V4[:, K * t : K * (t + 1)],
                        U[:, 128 * m + 32 * j : 128 * m + 32 * j + 32],
                        start=(t == 0),
                        stop=(t == NT - 1),
                    )
                # Mt: B-scaled moments
                Mt = wp.tile([K, C], BF16, tag="Mt")
                nc.vector.tensor_mul(
                    Mt[:], Mp[:], p1[0:K, P_B4 + C * i : P_B4 + C * (i + 1)]
                )
                # eval: PT[c + 32s, n'] = sum_k Mt[k,c] x_n^k, n = 512s + n'
                PT = ppc.tile([128, 512], F32, tag="PT")
                for s in range(4):
                    nc.tensor.matmul(
                        PT[32 * s : 32 * (s + 1), :],
                        Mt[:],
                        vt[:, 512 * s : 512 * (s + 1)],
                        start=True,
                        stop=True,
                        tile_position=(0, 32 * s),
                    )
                # combine: fq_next = gelu(skip + pw_b + Es * PT)
                z = wp.tile([128, 512], BF16, tag="z")
                nc.vector.tensor_mul(z[:], PT[:], Es(i))
                warm(z[:])
                if i < DEPTH - 1:
                    pre = wp.tile([128, 512], BF16, tag="pre")
                    nc.vector.tensor_add(pre[:], z[:], skp[:])
                    warm(pre[:])
                    fq = wp.tile([128, 512], BF16, tag="fq")
                    nc.scalar.activation(
                        fq[:], pre[:], AF.Gelu_apprx_tanh,
                        bias=pc[:, PC_PWB + i : PC_PWB + i + 1],
                    )
                else:
                    fq = wp.tile([128, 512], BF16, tag="fq")
                    nc.vector.scalar_tensor_tensor(
                        fq[:], z[:], pc[:, PC_PWB + i : PC_PWB + i + 1], skp[:],
                        ALU.add, ALU.add,
                    )

            # ---------------- projection head ----------------
            for woff, boff in ((P_P1W, PC_P1B), (P_P2W, PC_P2B)):
                ppj = ppa.tile([128, 512], F32, tag="mix")
                for j in range(NCHUNK):
                    nc.tensor.matmul(
                        ppj[32 * j : 32 * (j + 1), :],
                        p1[32 * j : 32 * (j + 1), woff : woff + C],
                        fq[32 * j : 32 * (j + 1), :],
                        start=True,
                        stop=True,
                        tile_position=(32 * j, 32 * j),
                    )
                nxt = wp.tile([128, 512], BF16, tag="fq")
                nc.scalar.activation(
                    nxt[:], ppj[:], AF.Gelu_apprx_tanh, bias=pc[:, boff : boff + 1]
                )
                fq = nxt

            p3p = ppc.tile([4, 512], F32, tag="PT")
            nc.tensor.matmul(
                p3p[:],
                p1[:, P_P3W : P_P3W + 4],
                fq[:],
                start=True,
                stop=True,
            )
            outsb = wp.tile([4, 512], F32, tag="outsb")
            nc.vector.tensor_scalar_add(
                outsb[:], p3p[:], pc[0:4, PC_P3B : PC_P3B + 1]
            )
            nc.sync.dma_start(
                out_dram[:].rearrange("o (b c) -> (o b) c", b=4), outsb[:]
            )

    return nc


def get_nc():
    if "nc" not in _CACHE:
        nc = bacc.Bacc("TRN2", target_bir_lowering=False, debug=False, num_devices=NCORES)
        build_program(nc)
        nc.compile()
        _CACHE["nc"] = nc
    return _CACHE["nc"]


def make_in_map(
    f_x, x_grid, q_weights, lift_W, lift_b, pw_W, pw_b, ker_log_ell, ker_log_sigma,
    proj1_W, proj1_b, proj2_W, proj2_b, proj3_W, proj3_b,
):
    f4 = lambda a: np.asarray(a, dtype=np.float64)
    x = f4(x_grid).reshape(N)
    w = f4(q_weights).reshape(N)
    ks = np.arange(K, dtype=np.float64)
    lnfact = np.concatenate([[0.0], np.cumsum(np.log(np.arange(1.0, K)))])
    a = 0.5 * np.exp(-2.0 * f4(ker_log_ell))        # [DEPTH, C]
    sig2 = np.exp(2.0 * f4(ker_log_sigma))          # [DEPTH, C]

    pack1 = np.zeros((128, W1), dtype=np.float32)
    pack1[:, P_IDENT : P_IDENT + 128] = np.eye(128, dtype=np.float32)
    # V4[p, 32t+k] = w_q x_q^k, q = 128t + p
    V = w[:, None] * np.power(x[:, None], ks[None, :])          # [N, K]
    pack1[:, P_V4 : P_V4 + NT * K] = (
        V.reshape(NT, 128, K).transpose(1, 0, 2).reshape(128, NT * K)
    ).astype(np.float32)
    # vt[k, n] = x_n^k
    VT = np.power(x[None, :], ks[:, None]).astype(np.float32)   # [K, N]
    # pwWT3[c+32j, 32i+c'] = pw_W[i][c', c]
    pwT = f4(pw_W).transpose(0, 2, 1)                           # [D, C, C']
    pack1[:, P_PWW : P_PWW + 3 * C] = np.tile(
        pwT.transpose(1, 0, 2).reshape(C, 3 * C), (4, 1)
    ).astype(np.float32)
    pack1[:, P_P1W : P_P1W + C] = np.tile(f4(proj1_W).T, (4, 1)).astype(np.float32)
    pack1[:, P_P2W : P_P2W + C] = np.tile(f4(proj2_W).T, (4, 1)).astype(np.float32)
    # B4_3[32s+k, 32i+c] = (2 a_ic)^k / k!
    B = np.exp(ks[None, :, None] * np.log(2.0 * a)[:, None, :] - lnfact[None, :, None])
    pack1[:, P_B4 : P_B4 + 3 * C] = np.tile(
        B.transpose(1, 0, 2).reshape(K, 3 * C), (4, 1)
    ).astype(np.float32)
    p3w4 = np.zeros((128, 4), dtype=np.float32)
    for j in range(4):
        p3w4[32 * j : 32 * (j + 1), j] = f4(proj3_W)[0]
    pack1[:, P_P3W : P_P3W + 4] = p3w4
    pack1[0:2, P_LW : P_LW + C] = f4(lift_W).T.astype(np.float32)
    packc = np.zeros((128, WC), dtype=np.float32)
    packc[:, PC_PWB : PC_PWB + 3] = np.tile(f4(pw_b).T, (4, 1)).astype(np.float32)
    packc[:, PC_P1B] = np.tile(f4(proj1_b), 4).astype(np.float32)
    packc[:, PC_P2B] = np.tile(f4(proj2_b), 4).astype(np.float32)
    packc[:, PC_P3B] = np.float32(f4(proj3_b)[0])
    packc[:, PC_LB] = np.tile(f4(lift_b), 4).astype(np.float32)

    # exponential prefactors per layer
    # tmpN[p, 128m+32j+c] = exp(-a_c x_q^2), q = 512j+128m+p
    # Es[c+32j, n'] = sig2_c exp(-a_c x_n^2), n = 512j+n'
    x2 = x * x
    packs = []
    for i in range(DEPTH):
        eN = np.exp(-x2[:, None] * a[i][None, :])               # [N, C]
        tm = (
            eN.reshape(NCHUNK, 4, 128, C).transpose(2, 1, 0, 3).reshape(128, 512)
        ).astype(np.float32)
        es = (
            (sig2[i][None, :] * eN).reshape(NCHUNK, 512, C)
            .transpose(0, 2, 1).reshape(128, 512)
        ).astype(np.float32)
        packs.append((tm, es))
    pack2 = np.concatenate(packs[0], axis=1)
    pack3 = np.concatenate([t for pr in packs[1:] for t in pr], axis=1)

    lift_arr = np.stack(
        [np.asarray(f_x, np.float32).reshape(N), x.astype(np.float32)]
    )
    return {
        "pack1": pack1.astype(NPBF16),
        "pack2": np.ascontiguousarray(pack2).astype(NPBF16),
        "pack3": np.ascontiguousarray(pack3).astype(NPBF16),
        "lift_in": np.ascontiguousarray(lift_arr).astype(NPBF16),
        "vt": np.ascontiguousarray(VT).astype(NPBF16),
        "packc": packc,
    }


def kernel(**inputs) -> np.ndarray:
    nc = get_nc()
    in_map = make_in_map(**inputs)
    res = run_bass_kernel_spmd(nc, [in_map] * NCORES, list(range(NCORES)))
    return np.asarray(res.results[0]["out"], dtype=np.float32).reshape(N)
